# revision 2
# baseline (speedup 1.0000x reference)
"""MultiHeadAttention forward on 8 Trainium2 NeuronCores — v2.

Problem: x[2,2048,1024] -> fused QKV proj -> 16-head attention -> out proj.
Sharding: (batch=2) x (head-groups=4) across 8 cores; core c: batch c//4,
heads 4g..4g+3 (g=c%4). Host sums the 4 head-group partial outputs per
batch and adds b_out once (row-parallel all-reduce equivalent).

Design notes (TimelineSim cost model: matmul engine time = N_stream_cols
x 0.4167ns regardless of K/M; ACT exp = cols x 0.833 + ~185ns/instr):
  - PV reoriented: stationary = exp-scores [k,q-tile] slice, stream =
    V (N=64) + ones (N=1 denominator column). Halves PV engine columns.
  - softmax normalize on DVE (denominator per q-partition), then
    [q,hd]->[hd,q] via DMA xbar transpose (no PE/PSUM cost).
  - out-proj bias on host; output stored bf16, host upcasts.
  - 8 streams (q2-half x head); exp on ACT is the pacer (~133us);
    projection/out-proj groups drip into PE idle via a deadline queue;
    PV of stream s drains during stream s+1 (1 ktile-pop per slot).
  - weights host-preshuffled to [P, ...] so every load is one
    contiguous-row DMA; lead-in keeps HWDGE exclusively for wqk02+xts
    token-half A, everything else on the SWDGE (gpsimd) path.
"""

import numpy as np
import ml_dtypes

import concourse.bass as bass
import concourse.bacc as bacc
import concourse.tile as tile
from concourse import mybir
from concourse.alu_op_type import AluOpType
from concourse.bass_utils import run_bass_kernel_spmd

BF16 = ml_dtypes.bfloat16

B, S, E = 2, 2048, 1024
H, D = 16, 64
HG = 4
N_CORES = 8
P = 128
ET = E // P        # 8 e-chunks
ST = S // P        # 16 k-tiles
NQ = 8             # q-tiles per q2-half

F32 = mybir.dt.float32
BF = mybir.dt.bfloat16
EXP = mybir.ActivationFunctionType.Exp
COPY = mybir.ActivationFunctionType.Copy

_COMPILED = None


def build_program():
    nc = bacc.Bacc("TRN2", target_bir_lowering=False, debug=False)

    xT_d = nc.dram_tensor("xT", [E, S], BF, kind="ExternalInput").ap()
    # host-preshuffled: row p holds e-major concat -> one contiguous DMA
    wqk02_d = nc.dram_tensor("wqk02", [P, ET * 2 * P], BF,
                             kind="ExternalInput").ap()
    wqk13_d = nc.dram_tensor("wqk13", [P, ET * 2 * P], BF,
                             kind="ExternalInput").ap()
    wv_d = nc.dram_tensor("wv", [P, ET * HG * D], BF,
                          kind="ExternalInput").ap()
    wout_d = nc.dram_tensor("wout", [P, 2 * E], BF, kind="ExternalInput").ap()
    bqk_d = nc.dram_tensor("bqk", [P, 4], F32, kind="ExternalInput").ap()
    bv_d = nc.dram_tensor("bv", [1, HG * D], F32, kind="ExternalInput").ap()
    out_d = nc.dram_tensor("out", [S, E], BF, kind="ExternalOutput").ap()

    with tile.TileContext(nc) as tc:
        with (
            tc.tile_pool(name="persist", bufs=1) as consts,
            tc.tile_pool(name="expp", bufs=36) as expp,
            tc.tile_pool(name="rcpp", bufs=2) as rcpp,
            tc.tile_pool(name="outsb", bufs=4) as outsb,
            tc.tile_pool(name="psS", bufs=2, space="PSUM") as psS,
            tc.tile_pool(name="psPV", bufs=2, space="PSUM") as psPV,
            tc.tile_pool(name="psD", bufs=1, space="PSUM") as psD,
            tc.tile_pool(name="psX", bufs=1, space="PSUM") as psX,
        ):
            # ---------------- input DMAs ---------------------------------
            # SP/HWDGE: the lead-in critical path only.
            wqk02 = consts.tile([P, ET, 2 * P], BF, tag="wqk02", name="wqk02")
            nc.sync.dma_start(wqk02, wqk02_d)
            xts = [consts.tile([P, S], BF, tag=f"xt{e}", name=f"xt{e}")
                   for e in range(ET)]
            for e in range(ET):
                nc.sync.dma_start(xts[e][:, 0:S // 2],
                                  xT_d[e * P:(e + 1) * P, 0:S // 2])
            for e in range(ET):
                nc.sync.dma_start(xts[e][:, S // 2:S],
                                  xT_d[e * P:(e + 1) * P, S // 2:S])
            # SWDGE (gpsimd): bqk + wv are needed early (~5-14us); the
            # rest is gated behind tiny Pool copies that wait for the
            # first pre evac, keeping the DMA bus free for the lead-in
            # xT transfers.
            bqk_sb = consts.tile([P, 4], F32, tag="bqk")
            nc.gpsimd.dma_start(bqk_sb, bqk_d)
            wv_all = consts.tile([P, ET, HG * D], BF, tag="wv", name="wv_all")
            wqk13 = consts.tile([P, ET, 2 * P], BF, tag="wqk13", name="wqk13")
            wout_all = consts.tile([P, 2, E], BF, tag="wout", name="wout_all")
            bv_bc = consts.tile([P, HG, D], F32, tag="bv")

            def late_weight_dmas(gate_src):
                # tiny writes into each tile create WAW deps so the big
                # loads can't be hoisted into the lead-in DMA window
                for w in (wv_all, wqk13, wout_all):
                    nc.gpsimd.tensor_copy(w[0:2, 0, 0:2], gate_src)
                nc.gpsimd.tensor_copy(
                    bv_bc[0:2, 0, 0:1], gate_src[:, 0:1])
                nc.gpsimd.dma_start(wv_all, wv_d)
                nc.gpsimd.dma_start(wqk13, wqk13_d)
                nc.gpsimd.dma_start(wout_all, wout_d)
                nc.gpsimd.dma_start(
                    bv_bc, bv_d.to_broadcast([P, HG * D]).rearrange(
                        "p (h d) -> p h d", h=HG))

            # ---------------- PE warmup ----------------------------------
            # dummy matmuls from t~0 so the p-state ramp (2x cost for the
            # first ~3us of PE activity) is spent before real work arrives
            wscr = consts.tile([P, 512], BF, tag="wscr")
            nc.vector.memset(wscr, 0.25)
            warm = psX.tile([P, 512], F32, tag="ps", name="warm")
            for _ in range(8):
                nc.tensor.matmul(warm, lhsT=wscr[:, 0:P], rhs=wscr,
                                 start=True, stop=True)

            # ---------------- persistent SBUF ----------------------------
            qkT = [consts.tile([P, S], BF, tag=f"qkT{m}", name=f"qkT{m}")
                   for m in range(4)]
            Vaug = [consts.tile([P, HG, 66], BF, tag=f"vaug{st}",
                                   name=f"vaug{st}") for st in range(ST)]
            for st in range(ST):
                nc.vector.memset(Vaug[st], 1.0)
            attnT = [consts.tile([P, S], BF, tag=f"attnT{c}", name=f"attnT{c}")
                     for c in range(2)]
            anrm = [consts.tile([P, NQ, P], BF, tag=f"anrm{i}",
                                   name=f"anrm{i}") for i in range(4)]
            # den bank: cols 0:16 = softmax denominators; cols 128:384 =
            # scratch psum for odd v-projection groups (double-buffers the
            # single psX slot so v-groups sustain 1-per-ktile)
            den_ps = psD.tile([P, 512], F32, tag="den", name="den_ps")
            nc.vector.memset(den_ps[:, 0:16], 0.0)

            wqk_at = {0: (wqk02, 0), 2: (wqk02, P), 1: (wqk13, 0),
                      3: (wqk13, P)}

            # ---------------- emission helpers ---------------------------
            def qk_group(m, s4):
                wt, co = wqk_at[m]
                ps = psX.tile([P, 512], F32, tag="ps", name=f"qk{m}_{s4}")
                for e in range(ET):
                    nc.tensor.matmul(
                        ps, lhsT=wt[:, e, co:co + P],
                        rhs=xts[e][:, s4 * 512:(s4 + 1) * 512],
                        start=(e == 0), stop=(e == ET - 1))
                nc.vector.tensor_scalar_add(
                    qkT[m][:, s4 * 512:(s4 + 1) * 512], ps, bqk_sb[:, m:m + 1])

            def v_evac(ps_ap, st):
                nc.vector.tensor_tensor(
                    Vaug[st][:, :, 0:D],
                    ps_ap.rearrange("p (h d) -> p h d", h=HG),
                    bv_bc, AluOpType.add)

            def v_fill(sts, use_den):
                # up to two k-tiles per psum region; den-bank scratch (256
                # cols at 16:272) takes singles, psX takes pairs — a
                # 2-deep chain so v-projection sustains ~1 tile/ktile
                if use_den:
                    # den bank is shared with live denominator columns:
                    # zero the scratch region with DVE, accumulate-only
                    # matmuls (start=True would wipe the whole bank)
                    ps = den_ps[:, 16:16 + 256 * len(sts)]
                    nc.vector.memset(ps, 0.0)
                else:
                    ps = psX.tile([P, 256 * len(sts)], F32, tag="ps",
                                  name=f"v{sts[0]}")
                for e in range(ET):
                    for j, st in enumerate(sts):
                        nc.tensor.matmul(
                            ps[:, j * 256:(j + 1) * 256],
                            lhsT=xts[e][:, st * P:(st + 1) * P],
                            rhs=wv_all[:, e, :],
                            start=(e == 0 and j == 0 and not use_den),
                            stop=(e == ET - 1),
                            skip_group_check=True)
                for j, st in enumerate(sts):
                    v_evac(ps[:, j * 256:(j + 1) * 256], st)

            ex_tiles = {}
            pv_ps = {}

            def scores_exp(s, k):
                q2, h = s // 4, s % 4
                pair, hp = h // 2, h % 2
                sc = psS.tile([P, 1024], F32, tag="sc", name=f"sc{s}_{k}")
                for j in range(2):      # matmul out must stay in one bank
                    nc.tensor.matmul(
                        sc[:, j * 512:(j + 1) * 512],
                        lhsT=qkT[2 + pair][hp * 64:(hp + 1) * 64,
                                           k * P:(k + 1) * P],
                        rhs=qkT[pair][hp * 64:(hp + 1) * 64,
                                      q2 * 1024 + j * 512:
                                      q2 * 1024 + (j + 1) * 512],
                        start=True, stop=True)
                ex = expp.tile([P, 1024], BF, tag="ex", name=f"ex{s}_{k}")
                nc.scalar.activation(ex, sc, EXP, scale=0.125)
                ex_tiles[(s, k)] = ex

            def pv(s, k):
                h, par = s % 4, s % 2
                if k == 0:
                    pv_ps[s] = psPV.tile([P, NQ, D], F32, tag="pv",
                                         name=f"pv{s}")
                ex = ex_tiles.pop((s, k))
                pvt = pv_ps[s]
                for qt in range(NQ):
                    lhsT = ex[:, qt * P:(qt + 1) * P]
                    # start=True zeroes the whole psum bank: emit it only
                    # on the tile's first matmul; den columns are zeroed
                    # by DVE memsets instead (bank shared across parities)
                    nc.tensor.matmul(
                        pvt[:, qt, :], lhsT=lhsT, rhs=Vaug[k][:, h, 0:D],
                        start=(k == 0 and qt == 0), stop=(k == ST - 1),
                        skip_group_check=True)
                    nc.tensor.matmul(
                        den_ps[:, par * 8 + qt:par * 8 + qt + 1],
                        lhsT=lhsT, rhs=Vaug[k][:, h, D:D + 1],
                        start=False, stop=(k == ST - 1),
                        skip_group_check=True)

            def norm(s):
                q2, h, par = s // 4, s % 4, s % 2
                pair, hp = h // 2, h % 2
                rcp = rcpp.tile([P, 8], F32, tag="rcp", name=f"rcp{s}")
                nc.vector.reciprocal(rcp, den_ps[:, par * 8:(par + 1) * 8])
                nc.vector.memset(den_ps[:, par * 8:(par + 1) * 8], 0.0)
                a = anrm[q2 * 2 + pair]
                pvt = pv_ps.pop(s)
                for qt in range(NQ):
                    nc.vector.tensor_scalar(
                        a[:, qt, hp * 64:(hp + 1) * 64], pvt[:, qt, :],
                        rcp[:, qt:qt + 1], None, AluOpType.mult)

            def tr(q2, c, qt):
                off = q2 * 1024 + qt * P
                nc.sync.dma_start_transpose(
                    attnT[c][:, off:off + P], anrm[q2 * 2 + c][:, qt, :])

            def outproj_mid(q2, qt, half):
                off = q2 * 1024 + qt * P
                ps = psX.tile([P, 512], F32, tag="ps",
                              name=f"op{q2}_{qt}_{half}")
                for c in range(2):
                    nc.tensor.matmul(
                        ps, lhsT=attnT[c][:, off:off + P],
                        rhs=wout_all[:, c, half * 512:(half + 1) * 512],
                        start=(c == 0), stop=(c == 1))
                ob = outsb.tile([P, 512], BF, tag="ob")
                nc.vector.tensor_copy(ob, ps)
                nc.gpsimd.dma_start(
                    out_d[off:off + P, half * 512:(half + 1) * 512], ob)

            def outproj_tail(q2, qt):
                off = q2 * 1024 + qt * P
                if qt % 2 == 0:
                    ps = psS.tile([P, 1024], F32, tag="sc", name=f"opt{qt}")
                    pss = [ps[:, 0:512], ps[:, 512:1024]]
                else:
                    pvt = psPV.tile([P, NQ, D], F32, tag="pv", name=f"opt{qt}")
                    pss = [pvt.rearrange("p a b -> p (a b)")[:, 0:512], None]
                    ps2 = psX.tile([P, 512], F32, tag="ps", name=f"optx{qt}")
                    pss[1] = ps2
                for half in range(2):
                    for c in range(2):
                        nc.tensor.matmul(
                            pss[half], lhsT=attnT[c][:, off:off + P],
                            rhs=wout_all[:, c, half * 512:(half + 1) * 512],
                            start=(c == 0), stop=(c == 1),
                            skip_group_check=True)
                ob = outsb.tile([P, 1024], BF, tag="ob2")
                if qt % 2 == 0:
                    nc.vector.tensor_copy(ob[:, 0:512], pss[0])
                    nc.vector.tensor_copy(ob[:, 512:1024], pss[1])
                else:
                    nc.scalar.activation(ob[:, 0:512], pss[0], COPY)
                    nc.scalar.activation(ob[:, 512:1024], pss[1], COPY)
                (nc.sync if qt % 2 else nc.scalar).dma_start(
                    out_d[off:off + P, :], ob)

            # ---------------- pre phase (e-major, xT-arrival paced) ------
            pre_ps = psS.tile([P, 1024], F32, tag="sc", name="pre_ps")
            pre_px = psX.tile([P, 512], F32, tag="ps", name="pre_px")
            for e in range(ET):
                st_ = (e == 0)
                sp_ = (e == ET - 1)
                nc.tensor.matmul(   # m2 s4=0 (k tokens 0:512)
                    pre_ps[:, 0:512], lhsT=wqk02[:, e, P:2 * P],
                    rhs=xts[e][:, 0:512], start=st_, stop=sp_)
                nc.tensor.matmul(   # m0 s4=0
                    pre_ps[:, 512:1024], lhsT=wqk02[:, e, 0:P],
                    rhs=xts[e][:, 0:512], start=st_, stop=sp_)
                nc.tensor.matmul(   # m0 s4=1
                    pre_px, lhsT=wqk02[:, e, 0:P],
                    rhs=xts[e][:, 512:1024], start=st_, stop=sp_)
            nc.vector.tensor_scalar_add(
                qkT[2][:, 0:512], pre_ps[:, 0:512], bqk_sb[:, 2:3])
            nc.vector.tensor_scalar_add(
                qkT[0][:, 0:512], pre_ps[:, 512:1024], bqk_sb[:, 0:1])
            nc.vector.tensor_scalar_add(
                qkT[0][:, 512:1024], pre_px, bqk_sb[:, 0:1])
            late_weight_dmas(qkT[2][0:2, 0:2])

            # ---------------- filler queue -------------------------------
            fillers = []
            for s4, dl in ((1, 2), (2, 5), (3, 9)):
                fillers.append((dl, 1706, lambda s4=s4: qk_group(2, s4)))
            # v coverage in [pair(psX), single(den)] groups; deadline =
            # first consumer slot (PV(0,st) pops at slot 22+st for st<=9,
            # 32+st-10 after) minus chain latency slack
            vsts = [((0, 1), False), ((2,), True), ((3, 4), False),
                    ((5,), True), ((6, 7), False), ((8,), True),
                    ((9, 10), False), ((11,), True), ((12, 13), False),
                    ((14,), True), ((15,), False)]
            for sts, use_den in vsts:
                st0 = sts[0]
                need = 22 + st0 if st0 <= 9 else 22 + st0
                fillers.append((need - 3, 853 * len(sts),
                                lambda sts=sts, d=use_den: v_fill(sts, d)))
            for s4, dl in ((0, 27), (1, 29)):
                fillers.append((dl, 1706, lambda s4=s4: qk_group(1, s4)))
            for s4, dl in ((0, 28), (1, 31), (2, 35), (3, 39)):
                fillers.append((dl, 1706, lambda s4=s4: qk_group(3, s4)))
            for s4, dl in ((2, 56), (3, 58)):
                fillers.append((dl, 1706, lambda s4=s4: qk_group(0, s4)))
            for s4, dl in ((2, 87), (3, 90)):
                fillers.append((dl, 1706, lambda s4=s4: qk_group(1, s4)))
            fillers.sort(key=lambda f: f[0])

            budget = [0.0]

            def drip(g, slot_budget):
                budget[0] = min(budget[0] + slot_budget, 1400.0)
                while fillers and (fillers[0][0] <= g + 1
                                   or budget[0] >= fillers[0][1]):
                    _, cost, fn = fillers.pop(0)
                    fn()
                    budget[0] -= cost

            # ---------------- streams ------------------------------------
            # PV(s) pops spread over stream s+1 kt6..15 and stream s+2
            # kt0..5, so v-projection deadlines land where PE has slack
            # and norms run mid-stream. Stream 7 compresses the schedule
            # to finish PV(6) by kt8 and PV(7) by stream end.
            pv_done = {}

            def pop_pv(s, i):
                pv(s, i)
                if i == ST - 1:
                    pv_done[s] = True
                    norm(s)
                    q2n, hn = s // 4, s % 4
                    if hn % 2 == 1:       # pair complete -> transposes
                        for qt in range(NQ):
                            tr(q2n, hn // 2, qt)

            for s in range(8):
                for k in range(ST):
                    g = s * ST + k
                    pv_load = 0
                    scores_exp(s, k)
                    if s < 7:
                        if k < 6 and s >= 2:
                            pop_pv(s - 2, 10 + k)
                            pv_load += 214
                        if k >= 6 and s >= 1:
                            pop_pv(s - 1, k - 6)
                            pv_load += 214
                    else:
                        if k < 6:
                            pop_pv(5, 10 + k)
                            pop_pv(6, 2 * k)
                            pop_pv(6, 2 * k + 1)
                            pv_load += 642
                        elif k < 8:
                            pop_pv(6, 2 * k)
                            pop_pv(6, 2 * k + 1)
                            pv_load += 428
                        else:
                            pop_pv(7, 2 * (k - 8))
                            pop_pv(7, 2 * (k - 8) + 1)
                            pv_load += 428
                    drip(g, 1038 - 427 - pv_load)
                if s == 5:
                    for qt in range(NQ):
                        for half in range(2):
                            i = qt * 2 + half
                            fillers.append(
                                (88 + (i * 24) // 16, 639,
                                 lambda qt=qt, half=half:
                                 outproj_mid(0, qt, half)))
                    fillers.sort(key=lambda f: f[0])

            # ---------------- tail ---------------------------------------
            # norm(7)+tr(1,1) were emitted by the final pop; outproj_tail
            # pipelines per-qt behind the transpose DMAs.
            while fillers:
                _, _, fn = fillers.pop(0)
                fn()
            for qt in range(NQ):
                outproj_tail(1, qt)

    nc.compile()
    return nc


def get_program():
    global _COMPILED
    if _COMPILED is None:
        _COMPILED = build_program()
    return _COMPILED


def _shuffle_pmajor(w, ncol):
    """[ET*P, ncol] -> [P, ET*ncol]: row p = concat_e w[e*P+p, :]."""
    return np.ascontiguousarray(
        w.reshape(ET, P, ncol).transpose(1, 0, 2).reshape(P, ET * ncol))


def make_in_maps(x, W_qkv, b_qkv, W_out, b_out):
    x = np.asarray(x, dtype=np.float32)
    W_qkv = np.asarray(W_qkv, dtype=np.float32)
    b_qkv = np.asarray(b_qkv, dtype=np.float32)
    W_out = np.asarray(W_out, dtype=np.float32)

    in_maps = []
    for c in range(N_CORES):
        b, g = c // 4, c % 4
        heads = [4 * g + i for i in range(HG)]
        xT = np.ascontiguousarray(x[b].T).astype(BF16)
        wq = np.empty((E, 4 * D), np.float32)
        wk = np.empty((E, 4 * D), np.float32)
        wv = np.empty((E, 4 * D), np.float32)
        bq = np.empty((4 * D,), np.float32)
        bk = np.empty((4 * D,), np.float32)
        bv = np.empty((1, 4 * D), np.float32)
        wout = np.empty((HG * D, E), np.float32)
        for i, h in enumerate(heads):
            base = h * 3 * D
            wq[:, i * D:(i + 1) * D] = W_qkv[:, base:base + D]
            wk[:, i * D:(i + 1) * D] = W_qkv[:, base + D:base + 2 * D]
            wv[:, i * D:(i + 1) * D] = W_qkv[:, base + 2 * D:base + 3 * D]
            bq[i * D:(i + 1) * D] = b_qkv[base:base + D]
            bk[i * D:(i + 1) * D] = b_qkv[base + D:base + 2 * D]
            bv[0, i * D:(i + 1) * D] = b_qkv[base + 2 * D:base + 3 * D]
            wout[i * D:(i + 1) * D, :] = W_out[h * D:(h + 1) * D, :]
        wqk02 = np.concatenate([wq[:, 0:P], wk[:, 0:P]], axis=1)
        wqk13 = np.concatenate([wq[:, P:2 * P], wk[:, P:2 * P]], axis=1)
        bqk = np.stack([bq[0:P], bq[P:2 * P], bk[0:P], bk[P:2 * P]], axis=1)
        wout_p = np.ascontiguousarray(    # [P, 2*E]: row p = [c0row, c1row]
            wout.reshape(2, P, E).transpose(1, 0, 2).reshape(P, 2 * E))
        in_maps.append({
            "xT": xT,
            "wqk02": _shuffle_pmajor(wqk02, 2 * P).astype(BF16),
            "wqk13": _shuffle_pmajor(wqk13, 2 * P).astype(BF16),
            "wv": _shuffle_pmajor(wv, HG * D).astype(BF16),
            "wout": wout_p.astype(BF16),
            "bqk": np.ascontiguousarray(bqk),
            "bv": bv,
        })
    return in_maps


def gather_outputs(results, b_out):
    out = np.zeros((B, S, E), np.float32)
    for c in range(N_CORES):
        out[c // 4] += np.asarray(results[c]["out"]).astype(np.float32)
    out += np.asarray(b_out, dtype=np.float32)
    return out


def run(in_maps, trace=False, **kwargs):
    nc = get_program()
    return run_bass_kernel_spmd(nc, in_maps, list(range(N_CORES)),
                                trace=trace, **kwargs)


def kernel(x, W_qkv, b_qkv, W_out, b_out):
    in_maps = make_in_maps(x, W_qkv, b_qkv, W_out, b_out)
    res = run(in_maps)
    return gather_outputs(res.results, b_out)


# revision 3
# speedup vs baseline: 1.0002x; 1.0002x over previous
"""MultiHeadAttention forward on 8 Trainium2 NeuronCores — v2.

Problem: x[2,2048,1024] -> fused QKV proj -> 16-head attention -> out proj.
Sharding: (batch=2) x (head-groups=4) across 8 cores; core c: batch c//4,
heads 4g..4g+3 (g=c%4). Host sums the 4 head-group partial outputs per
batch and adds b_out once (row-parallel all-reduce equivalent).

Design notes (TimelineSim cost model: matmul engine time = N_stream_cols
x 0.4167ns regardless of K/M; ACT exp = cols x 0.833 + ~185ns/instr):
  - PV reoriented: stationary = exp-scores [k,q-tile] slice, stream =
    V (N=64) + ones (N=1 denominator column). Halves PV engine columns.
  - softmax normalize on DVE (denominator per q-partition), then
    [q,hd]->[hd,q] via DMA xbar transpose (no PE/PSUM cost).
  - out-proj bias on host; output stored bf16, host upcasts.
  - 8 streams (q2-half x head); exp on ACT is the pacer (~133us);
    projection/out-proj groups drip into PE idle via a deadline queue;
    PV of stream s drains during stream s+1 (1 ktile-pop per slot).
  - weights host-preshuffled to [P, ...] so every load is one
    contiguous-row DMA; lead-in keeps HWDGE exclusively for wqk02+xts
    token-half A, everything else on the SWDGE (gpsimd) path.
"""

import numpy as np
import ml_dtypes

import concourse.bass as bass
import concourse.bacc as bacc
import concourse.tile as tile
from concourse import mybir
from concourse.alu_op_type import AluOpType
from concourse.bass_utils import run_bass_kernel_spmd

BF16 = ml_dtypes.bfloat16

B, S, E = 2, 2048, 1024
H, D = 16, 64
HG = 4
N_CORES = 8
P = 128
ET = E // P        # 8 e-chunks
ST = S // P        # 16 k-tiles
NQ = 8             # q-tiles per q2-half

F32 = mybir.dt.float32
BF = mybir.dt.bfloat16
EXP = mybir.ActivationFunctionType.Exp
COPY = mybir.ActivationFunctionType.Copy

_COMPILED = None


def build_program():
    nc = bacc.Bacc("TRN2", target_bir_lowering=False, debug=False)

    xT_d = nc.dram_tensor("xT", [E, S], BF, kind="ExternalInput").ap()
    # host-preshuffled: row p holds e-major concat -> one contiguous DMA
    wqk02_d = nc.dram_tensor("wqk02", [P, ET * 2 * P], BF,
                             kind="ExternalInput").ap()
    wqk13_d = nc.dram_tensor("wqk13", [P, ET * 2 * P], BF,
                             kind="ExternalInput").ap()
    wv_d = nc.dram_tensor("wv", [P, ET * HG * D], BF,
                          kind="ExternalInput").ap()
    wout_d = nc.dram_tensor("wout", [P, 2 * E], BF, kind="ExternalInput").ap()
    bqk_d = nc.dram_tensor("bqk", [P, 4], F32, kind="ExternalInput").ap()
    bv_d = nc.dram_tensor("bv", [1, HG * D], F32, kind="ExternalInput").ap()
    out_d = nc.dram_tensor("out", [S, E], BF, kind="ExternalOutput").ap()

    with tile.TileContext(nc) as tc:
        with (
            tc.tile_pool(name="persist", bufs=1) as consts,
            tc.tile_pool(name="expp", bufs=36) as expp,
            tc.tile_pool(name="rcpp", bufs=2) as rcpp,
            tc.tile_pool(name="outsb", bufs=4) as outsb,
            tc.tile_pool(name="psS", bufs=2, space="PSUM") as psS,
            tc.tile_pool(name="psPV", bufs=2, space="PSUM") as psPV,
            tc.tile_pool(name="psD", bufs=1, space="PSUM") as psD,
            tc.tile_pool(name="psX", bufs=1, space="PSUM") as psX,
        ):
            # ---------------- input DMAs ---------------------------------
            # SP/HWDGE: the lead-in critical path only.
            wqk02 = consts.tile([P, ET, 2 * P], BF, tag="wqk02", name="wqk02")
            nc.sync.dma_start(wqk02, wqk02_d)
            xts = [consts.tile([P, S], BF, tag=f"xt{e}", name=f"xt{e}")
                   for e in range(ET)]
            for e in range(ET):
                nc.sync.dma_start(xts[e][:, 0:S // 2],
                                  xT_d[e * P:(e + 1) * P, 0:S // 2])
            for e in range(ET):
                nc.sync.dma_start(xts[e][:, S // 2:S],
                                  xT_d[e * P:(e + 1) * P, S // 2:S])
            # SWDGE (gpsimd): bqk + wv are needed early (~5-14us); the
            # rest is gated behind tiny Pool copies that wait for the
            # first pre evac, keeping the DMA bus free for the lead-in
            # xT transfers.
            bqk_sb = consts.tile([P, 4], F32, tag="bqk")
            nc.gpsimd.dma_start(bqk_sb, bqk_d)
            wv_all = consts.tile([P, ET, HG * D], BF, tag="wv", name="wv_all")
            wqk13 = consts.tile([P, ET, 2 * P], BF, tag="wqk13", name="wqk13")
            wout_all = consts.tile([P, 2, E], BF, tag="wout", name="wout_all")
            bv_bc = consts.tile([P, HG, D], F32, tag="bv")

            def late_weight_dmas(gate_src):
                # tiny writes into each tile create WAW deps so the big
                # loads can't be hoisted into the lead-in DMA window
                for w in (wv_all, wqk13, wout_all):
                    nc.gpsimd.tensor_copy(w[0:2, 0, 0:2], gate_src)
                nc.gpsimd.tensor_copy(
                    bv_bc[0:2, 0, 0:1], gate_src[:, 0:1])
                nc.gpsimd.dma_start(wv_all, wv_d)
                nc.gpsimd.dma_start(wqk13, wqk13_d)
                nc.gpsimd.dma_start(wout_all, wout_d)
                nc.gpsimd.dma_start(
                    bv_bc, bv_d.to_broadcast([P, HG * D]).rearrange(
                        "p (h d) -> p h d", h=HG))

            # ---------------- PE warmup ----------------------------------
            # dummy matmuls from t~0 so the p-state ramp (2x cost for the
            # first ~3us of PE activity) is spent before real work arrives
            wscr = consts.tile([P, 512], BF, tag="wscr")
            nc.vector.memset(wscr, 0.25)
            warm = psX.tile([P, 512], F32, tag="ps", name="warm")
            for _ in range(8):
                nc.tensor.matmul(warm, lhsT=wscr[:, 0:P], rhs=wscr,
                                 start=True, stop=True)

            # ---------------- persistent SBUF ----------------------------
            qkT = [consts.tile([P, S], BF, tag=f"qkT{m}", name=f"qkT{m}")
                   for m in range(4)]
            Vaug = [consts.tile([P, HG, 66], BF, tag=f"vaug{st}",
                                   name=f"vaug{st}") for st in range(ST)]
            for st in range(ST):
                nc.vector.memset(Vaug[st], 1.0)
            attnT = [consts.tile([P, S], BF, tag=f"attnT{c}", name=f"attnT{c}")
                     for c in range(2)]
            anrm = [consts.tile([P, NQ, P], BF, tag=f"anrm{i}",
                                   name=f"anrm{i}") for i in range(4)]
            # den bank: cols 0:16 = softmax denominators; cols 128:384 =
            # scratch psum for odd v-projection groups (double-buffers the
            # single psX slot so v-groups sustain 1-per-ktile)
            den_ps = psD.tile([P, 512], F32, tag="den", name="den_ps")
            nc.vector.memset(den_ps[:, 0:16], 0.0)

            wqk_at = {0: (wqk02, 0), 2: (wqk02, P), 1: (wqk13, 0),
                      3: (wqk13, P)}

            # ---------------- emission helpers ---------------------------
            def qk_group(m, s4):
                wt, co = wqk_at[m]
                ps = psX.tile([P, 512], F32, tag="ps", name=f"qk{m}_{s4}")
                for e in range(ET):
                    nc.tensor.matmul(
                        ps, lhsT=wt[:, e, co:co + P],
                        rhs=xts[e][:, s4 * 512:(s4 + 1) * 512],
                        start=(e == 0), stop=(e == ET - 1))
                nc.vector.tensor_scalar_add(
                    qkT[m][:, s4 * 512:(s4 + 1) * 512], ps, bqk_sb[:, m:m + 1])

            def v_evac(ps_ap, st):
                nc.vector.tensor_tensor(
                    Vaug[st][:, :, 0:D],
                    ps_ap.rearrange("p (h d) -> p h d", h=HG),
                    bv_bc, AluOpType.add)

            def v_fill(sts, use_den):
                # up to two k-tiles per psum region; den-bank scratch (256
                # cols at 16:272) takes singles, psX takes pairs — a
                # 2-deep chain so v-projection sustains ~1 tile/ktile
                if use_den:
                    # den bank is shared with live denominator columns:
                    # zero the scratch region with DVE, accumulate-only
                    # matmuls (start=True would wipe the whole bank)
                    ps = den_ps[:, 16:16 + 256 * len(sts)]
                    nc.vector.memset(ps, 0.0)
                else:
                    ps = psX.tile([P, 256 * len(sts)], F32, tag="ps",
                                  name=f"v{sts[0]}")
                for e in range(ET):
                    for j, st in enumerate(sts):
                        nc.tensor.matmul(
                            ps[:, j * 256:(j + 1) * 256],
                            lhsT=xts[e][:, st * P:(st + 1) * P],
                            rhs=wv_all[:, e, :],
                            start=(e == 0 and j == 0 and not use_den),
                            stop=(e == ET - 1),
                            skip_group_check=True)
                for j, st in enumerate(sts):
                    v_evac(ps[:, j * 256:(j + 1) * 256], st)

            ex_tiles = {}
            pv_ps = {}

            def scores_exp(s, k):
                q2, h = s // 4, s % 4
                pair, hp = h // 2, h % 2
                sc = psS.tile([P, 1024], F32, tag="sc", name=f"sc{s}_{k}")
                for j in range(2):      # matmul out must stay in one bank
                    nc.tensor.matmul(
                        sc[:, j * 512:(j + 1) * 512],
                        lhsT=qkT[2 + pair][hp * 64:(hp + 1) * 64,
                                           k * P:(k + 1) * P],
                        rhs=qkT[pair][hp * 64:(hp + 1) * 64,
                                      q2 * 1024 + j * 512:
                                      q2 * 1024 + (j + 1) * 512],
                        start=True, stop=True)
                ex = expp.tile([P, 1024], BF, tag="ex", name=f"ex{s}_{k}")
                nc.scalar.activation(ex, sc, EXP, scale=0.125)
                ex_tiles[(s, k)] = ex

            def pv(s, k):
                h, par = s % 4, s % 2
                if k == 0:
                    pv_ps[s] = psPV.tile([P, NQ, D], F32, tag="pv",
                                         name=f"pv{s}")
                ex = ex_tiles.pop((s, k))
                pvt = pv_ps[s]
                for qt in range(NQ):
                    lhsT = ex[:, qt * P:(qt + 1) * P]
                    # start=True zeroes the whole psum bank: emit it only
                    # on the tile's first matmul; den columns are zeroed
                    # by DVE memsets instead (bank shared across parities)
                    nc.tensor.matmul(
                        pvt[:, qt, :], lhsT=lhsT, rhs=Vaug[k][:, h, 0:D],
                        start=(k == 0 and qt == 0), stop=(k == ST - 1),
                        skip_group_check=True)
                    nc.tensor.matmul(
                        den_ps[:, par * 8 + qt:par * 8 + qt + 1],
                        lhsT=lhsT, rhs=Vaug[k][:, h, D:D + 1],
                        start=False, stop=(k == ST - 1),
                        skip_group_check=True)

            def norm(s):
                q2, h, par = s // 4, s % 4, s % 2
                pair, hp = h // 2, h % 2
                rcp = rcpp.tile([P, 8], F32, tag="rcp", name=f"rcp{s}")
                nc.vector.reciprocal(rcp, den_ps[:, par * 8:(par + 1) * 8])
                nc.vector.memset(den_ps[:, par * 8:(par + 1) * 8], 0.0)
                a = anrm[q2 * 2 + pair]
                pvt = pv_ps.pop(s)
                for qt in range(NQ):
                    nc.vector.tensor_scalar(
                        a[:, qt, hp * 64:(hp + 1) * 64], pvt[:, qt, :],
                        rcp[:, qt:qt + 1], None, AluOpType.mult)

            def tr(q2, c, qt):
                off = q2 * 1024 + qt * P
                nc.sync.dma_start_transpose(
                    attnT[c][:, off:off + P], anrm[q2 * 2 + c][:, qt, :])

            def outproj_mid(q2, qt, half):
                off = q2 * 1024 + qt * P
                ps = psX.tile([P, 512], F32, tag="ps",
                              name=f"op{q2}_{qt}_{half}")
                for c in range(2):
                    nc.tensor.matmul(
                        ps, lhsT=attnT[c][:, off:off + P],
                        rhs=wout_all[:, c, half * 512:(half + 1) * 512],
                        start=(c == 0), stop=(c == 1))
                ob = outsb.tile([P, 512], BF, tag="ob")
                nc.vector.tensor_copy(ob, ps)
                nc.gpsimd.dma_start(
                    out_d[off:off + P, half * 512:(half + 1) * 512], ob)

            def outproj_tail(q2, qt):
                off = q2 * 1024 + qt * P
                if qt % 2 == 0:
                    ps = psS.tile([P, 1024], F32, tag="sc", name=f"opt{qt}")
                    pss = [ps[:, 0:512], ps[:, 512:1024]]
                else:
                    pvt = psPV.tile([P, NQ, D], F32, tag="pv", name=f"opt{qt}")
                    pss = [pvt.rearrange("p a b -> p (a b)")[:, 0:512], None]
                    ps2 = psX.tile([P, 512], F32, tag="ps", name=f"optx{qt}")
                    pss[1] = ps2
                for half in range(2):
                    for c in range(2):
                        nc.tensor.matmul(
                            pss[half], lhsT=attnT[c][:, off:off + P],
                            rhs=wout_all[:, c, half * 512:(half + 1) * 512],
                            start=(c == 0), stop=(c == 1),
                            skip_group_check=True)
                ob = outsb.tile([P, 1024], BF, tag="ob2")
                if qt % 2 == 0:
                    nc.vector.tensor_copy(ob[:, 0:512], pss[0])
                    nc.vector.tensor_copy(ob[:, 512:1024], pss[1])
                else:
                    nc.scalar.activation(ob[:, 0:512], pss[0], COPY)
                    nc.scalar.activation(ob[:, 512:1024], pss[1], COPY)
                (nc.sync if qt % 2 else nc.scalar).dma_start(
                    out_d[off:off + P, :], ob)

            # ---------------- pre phase (e-major, xT-arrival paced) ------
            pre_ps = psS.tile([P, 1024], F32, tag="sc", name="pre_ps")
            pre_pb = psS.tile([P, 1024], F32, tag="sc", name="pre_pb")
            pre_px = psX.tile([P, 512], F32, tag="ps", name="pre_px")
            for e in range(ET):
                st_ = (e == 0)
                sp_ = (e == ET - 1)
                nc.tensor.matmul(   # m2 s4=0 (k tokens 0:512)
                    pre_ps[:, 0:512], lhsT=wqk02[:, e, P:2 * P],
                    rhs=xts[e][:, 0:512], start=st_, stop=sp_)
                nc.tensor.matmul(   # m0 s4=0
                    pre_ps[:, 512:1024], lhsT=wqk02[:, e, 0:P],
                    rhs=xts[e][:, 0:512], start=st_, stop=sp_)
                nc.tensor.matmul(   # m0 s4=1
                    pre_px, lhsT=wqk02[:, e, 0:P],
                    rhs=xts[e][:, 512:1024], start=st_, stop=sp_)
                nc.tensor.matmul(   # m2 s4=1 (k tokens 512:1024)
                    pre_pb[:, 0:512], lhsT=wqk02[:, e, P:2 * P],
                    rhs=xts[e][:, 512:1024], start=st_, stop=sp_)
            nc.vector.tensor_scalar_add(
                qkT[2][:, 0:512], pre_ps[:, 0:512], bqk_sb[:, 2:3])
            nc.vector.tensor_scalar_add(
                qkT[0][:, 0:512], pre_ps[:, 512:1024], bqk_sb[:, 0:1])
            nc.vector.tensor_scalar_add(
                qkT[0][:, 512:1024], pre_px, bqk_sb[:, 0:1])
            nc.vector.tensor_scalar_add(
                qkT[2][:, 512:1024], pre_pb[:, 0:512], bqk_sb[:, 2:3])
            late_weight_dmas(qkT[2][0:2, 0:2])

            # ---------------- filler queue -------------------------------
            fillers = []
            for s4, dl in ((2, 5), (3, 9)):
                fillers.append((dl, 1706, lambda s4=s4: qk_group(2, s4)))
            # v coverage in [pair(psX), single(den)] groups; deadline =
            # first consumer slot (PV(0,st) pops at slot 22+st for st<=9,
            # 32+st-10 after) minus chain latency slack
            vsts = [((0, 1), False), ((2,), True), ((3, 4), False),
                    ((5,), True), ((6, 7), False), ((8,), True),
                    ((9, 10), False), ((11,), True), ((12, 13), False),
                    ((14,), True), ((15,), False)]
            for sts, use_den in vsts:
                st0 = sts[0]
                need = 22 + st0 if st0 <= 9 else 22 + st0
                fillers.append((need - 3, 853 * len(sts),
                                lambda sts=sts, d=use_den: v_fill(sts, d)))
            for s4, dl in ((0, 27), (1, 29)):
                fillers.append((dl, 1706, lambda s4=s4: qk_group(1, s4)))
            for s4, dl in ((0, 28), (1, 31), (2, 35), (3, 39)):
                fillers.append((dl, 1706, lambda s4=s4: qk_group(3, s4)))
            for s4, dl in ((2, 56), (3, 58)):
                fillers.append((dl, 1706, lambda s4=s4: qk_group(0, s4)))
            for s4, dl in ((2, 87), (3, 90)):
                fillers.append((dl, 1706, lambda s4=s4: qk_group(1, s4)))
            fillers.sort(key=lambda f: f[0])

            budget = [0.0]

            def drip(g, slot_budget):
                budget[0] = min(budget[0] + slot_budget, 1400.0)
                while fillers and (fillers[0][0] <= g + 1
                                   or budget[0] >= fillers[0][1]):
                    _, cost, fn = fillers.pop(0)
                    fn()
                    budget[0] -= cost

            # ---------------- streams ------------------------------------
            # PV(s) pops spread over stream s+1 kt6..15 and stream s+2
            # kt0..5, so v-projection deadlines land where PE has slack
            # and norms run mid-stream. Stream 7 compresses the schedule
            # to finish PV(6) by kt8 and PV(7) by stream end.
            pv_done = {}

            def pop_pv(s, i):
                pv(s, i)
                if i == ST - 1:
                    pv_done[s] = True
                    norm(s)
                    q2n, hn = s // 4, s % 4
                    if hn % 2 == 1:       # pair complete -> transposes
                        for qt in range(NQ):
                            tr(q2n, hn // 2, qt)

            for s in range(8):
                for k in range(ST):
                    g = s * ST + k
                    pv_load = 0
                    scores_exp(s, k)
                    if s < 7:
                        if k < 6 and s >= 2:
                            pop_pv(s - 2, 10 + k)
                            pv_load += 214
                        if k >= 6 and s >= 1:
                            pop_pv(s - 1, k - 6)
                            pv_load += 214
                    else:
                        if k < 6:
                            pop_pv(5, 10 + k)
                            pop_pv(6, 2 * k)
                            pop_pv(6, 2 * k + 1)
                            pv_load += 642
                        elif k < 8:
                            pop_pv(6, 2 * k)
                            pop_pv(6, 2 * k + 1)
                            pv_load += 428
                        else:
                            pop_pv(7, 2 * (k - 8))
                            pop_pv(7, 2 * (k - 8) + 1)
                            pv_load += 428
                    drip(g, 1038 - 427 - pv_load)
                if s == 5:
                    for qt in range(NQ):
                        for half in range(2):
                            i = qt * 2 + half
                            fillers.append(
                                (88 + (i * 24) // 16, 639,
                                 lambda qt=qt, half=half:
                                 outproj_mid(0, qt, half)))
                    fillers.sort(key=lambda f: f[0])

            # ---------------- tail ---------------------------------------
            # norm(7)+tr(1,1) were emitted by the final pop; outproj_tail
            # pipelines per-qt behind the transpose DMAs.
            while fillers:
                _, _, fn = fillers.pop(0)
                fn()
            for qt in range(NQ):
                outproj_tail(1, qt)

    nc.compile()
    return nc


def get_program():
    global _COMPILED
    if _COMPILED is None:
        _COMPILED = build_program()
    return _COMPILED


def _shuffle_pmajor(w, ncol):
    """[ET*P, ncol] -> [P, ET*ncol]: row p = concat_e w[e*P+p, :]."""
    return np.ascontiguousarray(
        w.reshape(ET, P, ncol).transpose(1, 0, 2).reshape(P, ET * ncol))


def make_in_maps(x, W_qkv, b_qkv, W_out, b_out):
    x = np.asarray(x, dtype=np.float32)
    W_qkv = np.asarray(W_qkv, dtype=np.float32)
    b_qkv = np.asarray(b_qkv, dtype=np.float32)
    W_out = np.asarray(W_out, dtype=np.float32)

    in_maps = []
    for c in range(N_CORES):
        b, g = c // 4, c % 4
        heads = [4 * g + i for i in range(HG)]
        xT = np.ascontiguousarray(x[b].T).astype(BF16)
        wq = np.empty((E, 4 * D), np.float32)
        wk = np.empty((E, 4 * D), np.float32)
        wv = np.empty((E, 4 * D), np.float32)
        bq = np.empty((4 * D,), np.float32)
        bk = np.empty((4 * D,), np.float32)
        bv = np.empty((1, 4 * D), np.float32)
        wout = np.empty((HG * D, E), np.float32)
        for i, h in enumerate(heads):
            base = h * 3 * D
            wq[:, i * D:(i + 1) * D] = W_qkv[:, base:base + D]
            wk[:, i * D:(i + 1) * D] = W_qkv[:, base + D:base + 2 * D]
            wv[:, i * D:(i + 1) * D] = W_qkv[:, base + 2 * D:base + 3 * D]
            bq[i * D:(i + 1) * D] = b_qkv[base:base + D]
            bk[i * D:(i + 1) * D] = b_qkv[base + D:base + 2 * D]
            bv[0, i * D:(i + 1) * D] = b_qkv[base + 2 * D:base + 3 * D]
            wout[i * D:(i + 1) * D, :] = W_out[h * D:(h + 1) * D, :]
        wqk02 = np.concatenate([wq[:, 0:P], wk[:, 0:P]], axis=1)
        wqk13 = np.concatenate([wq[:, P:2 * P], wk[:, P:2 * P]], axis=1)
        bqk = np.stack([bq[0:P], bq[P:2 * P], bk[0:P], bk[P:2 * P]], axis=1)
        wout_p = np.ascontiguousarray(    # [P, 2*E]: row p = [c0row, c1row]
            wout.reshape(2, P, E).transpose(1, 0, 2).reshape(P, 2 * E))
        in_maps.append({
            "xT": xT,
            "wqk02": _shuffle_pmajor(wqk02, 2 * P).astype(BF16),
            "wqk13": _shuffle_pmajor(wqk13, 2 * P).astype(BF16),
            "wv": _shuffle_pmajor(wv, HG * D).astype(BF16),
            "wout": wout_p.astype(BF16),
            "bqk": np.ascontiguousarray(bqk),
            "bv": bv,
        })
    return in_maps


def gather_outputs(results, b_out):
    out = np.zeros((B, S, E), np.float32)
    for c in range(N_CORES):
        out[c // 4] += np.asarray(results[c]["out"]).astype(np.float32)
    out += np.asarray(b_out, dtype=np.float32)
    return out


def run(in_maps, trace=False, **kwargs):
    nc = get_program()
    return run_bass_kernel_spmd(nc, in_maps, list(range(N_CORES)),
                                trace=trace, **kwargs)


def kernel(x, W_qkv, b_qkv, W_out, b_out):
    in_maps = make_in_maps(x, W_qkv, b_qkv, W_out, b_out)
    res = run(in_maps)
    return gather_outputs(res.results, b_out)
